# revision 1
# baseline (speedup 1.0000x reference)
"""Multi-head attention (B=2, S=2048, D=1024, H=16) on 8 TRN2 NeuronCores.

Sharding: core c handles batch b = c//4 and head-group g = c%4 (4 heads,
d-slice of 256). Host compacts keys/values by the attention mask (exact:
masked keys contribute exp->0 in the fp32 reference), pads to a multiple
of 128; a valid-flag column excludes padding from numerator/denominator.

Per core (bf16 matmuls throughout, fp32 PSUM accumulation):
  Q^T = WqT.T @ X^T (+bq)          [256, 2048]
  K^T = WkT.T @ Xkv^T (+bk)        [256, SKV]
  V   = Xkv^T-chunks @ WvT         [SKV, 4h, 64+vf]
  per (h, q-block): S^T = K_h Q_h^T, P = exp(S^T/8) on Act (bf16 out)
  psO[65, 512] accumulates [V_h | vf].T @ P over key chunks:
     rows 0..63 = unnormalized O^T, row 64 = softmax denominator
  O^T = psO[0:64] * recip(den)  (DVE recip, GpSimd partition-broadcast)
  OUT_partial = O^T.T @ WoT        [2048, 1024], bf16 out

The (qb, h) loop is software-pipelined: Q-projection for block qb+1 and
the out-projection for block qb-1 are emitted inside block qb's head
slots, with the A.V matmuls sandwiched between score groups so the
in-order PE queue never head-of-line blocks on an exp. PSUM: scores/exp
ring is 2 slots of [128,2,512] (tag s2, 4 banks), everything else
rotates through 4 single-bank b1 slots. Input DMAs are ordered by
criticality and split across the SP and GpSimd descriptor queues.

V/O biases fold into a host-side constant: A@(V+bv)Wo^T + bo =
A@V@Wo^T + (bv@Wo^T + bo). Partial outputs over head-groups are summed
on the host.
"""

import math
import os
from functools import lru_cache

import numpy as np

D_MODEL = 1024
NUM_HEADS = 16
D_K = 64


class _ActCopy:
    """tensor_copy shim routing through the Act engine's activation-Copy."""

    def __init__(self, nc):
        self.nc = nc

    def tensor_copy(self, out, in_):
        self.nc.scalar.copy(out, in_)
B = 2
S = 2048
N_CORES = 8
GROUPS = 4          # head-groups = cores per batch
DH = 256            # d-slice per core (4 heads x 64)
NH_LOC = 4          # heads per core
P = 128
CC = D_MODEL // P   # contraction chunks (8)

# results of the last hardware run (BassKernelResults), for test harnesses
last_results = None


@lru_cache(maxsize=2)
def _build(SKV: int):
    import concourse.mybir as mybir
    import concourse.tile as tile
    from concourse import bacc

    f32 = mybir.dt.float32
    bf16 = mybir.dt.bfloat16
    fp8 = mybir.dt.float8e4
    DR = mybir.MatmulPerfMode.DoubleRow
    KC = SKV // P
    QB = S // 512                       # q blocks of 512
    NSLOT = QB * NH_LOC
    NCP = CC // 2                       # cc pairs (4)
    kc_pairs = [(k, min(2, KC - k)) for k in range(0, KC, 2)]
    kc_groups = [list(range(g, min(g + 2, KC))) for g in range(0, KC, 2)]
    kb512 = [(s0, min(512, SKV - s0)) for s0 in range(0, SKV, 512)]
    assert len(kb512) <= 3, f"SKV={SKV} too large for single K psum slot"

    nc = bacc.Bacc("TRN2", target_bir_lowering=False, debug=False,
                   num_devices=N_CORES)

    XT_d = nc.dram_tensor("xt", [D_MODEL, S], bf16, kind="ExternalInput").ap()
    XKV_d = nc.dram_tensor("xkv", [D_MODEL, SKV], bf16, kind="ExternalInput").ap()
    WQT_d = nc.dram_tensor("wqt", [D_MODEL, DH], bf16, kind="ExternalInput").ap()
    WKT_d = nc.dram_tensor("wkt", [D_MODEL, DH], bf16, kind="ExternalInput").ap()
    WVT_d = nc.dram_tensor("wvt", [D_MODEL, DH], bf16, kind="ExternalInput").ap()
    WOT_d = nc.dram_tensor("wot", [DH, D_MODEL], bf16, kind="ExternalInput").ap()
    bq_d = nc.dram_tensor("bq", [DH], f32, kind="ExternalInput").ap()
    bk_d = nc.dram_tensor("bk", [DH], f32, kind="ExternalInput").ap()
    vf_d = nc.dram_tensor("vf", [SKV], bf16, kind="ExternalInput").ap()
    OUT_d = nc.dram_tensor("out", [S, D_MODEL], bf16, kind="ExternalOutput").ap()

    with tile.TileContext(nc) as tc:
        with tc.tile_pool(name="res", bufs=1) as res, \
             tc.tile_pool(name="ps", bufs=2, space="PSUM") as ps, \
             tc.tile_pool(name="qtp", bufs=4) as qtp, \
             tc.tile_pool(name="ptp", bufs=4) as ptp, \
             tc.tile_pool(name="otp", bufs=4) as otp, \
             tc.tile_pool(name="nrm", bufs=6) as nrm:
            XKV_sb = res.tile([P, CC, SKV], bf16)
            WKT_sb = res.tile([P, CC, DH], bf16)
            WVT_sb = res.tile([P, CC, DH], bf16)
            WQT_sb = res.tile([P, CC, DH], bf16)
            XT_sb = res.tile([P, CC, S], bf16)
            WOT_sb = res.tile([P, 2, D_MODEL], bf16)
            bq_sb = res.tile([P, 2], f32)
            bk_sb = res.tile([P, 2], f32)
            KT_sb = res.tile([P, 2, SKV], bf16)
            # row width padded 65 -> 128: dual-fp8 ldweights requires the
            # pair stride to be a multiple of 128 bytes
            V_sb = res.tile([P, KC, NH_LOC, 65], bf16)

            # input DMAs, issued across engines so descriptor generation
            # and transfers run in parallel queues; critical path first
            # (K-proj needs XKV+WKT, then Q-proj(qb0) needs XT0+WQT)
            nc.sync.dma_start(WKT_sb[:], WKT_d.rearrange("(c p) d -> p c d", p=P))
            for cq in range(0, 4, 2):
                nc.sync.dma_start(XKV_sb[:, cq:cq + 2, :],
                                  XKV_d.rearrange("(c p) k -> p c k", p=P)
                                      [:, cq:cq + 2, :])
            for cq in range(4, 8, 2):
                nc.sync.dma_start(XKV_sb[:, cq:cq + 2, :],
                                  XKV_d.rearrange("(c p) k -> p c k", p=P)
                                      [:, cq:cq + 2, :])
            nc.sync.dma_start(WQT_sb[:], WQT_d.rearrange("(c p) d -> p c d", p=P))
            nc.sync.dma_start(
                XT_sb[:, :, 0:512],
                XT_d.rearrange("(c p) q -> p c q", p=P)[:, :, 0:512])
            nc.sync.dma_start(bq_sb[:], bq_d.rearrange("(t p) -> p t", p=P))
            nc.sync.dma_start(bk_sb[:], bk_d.rearrange("(t p) -> p t", p=P))
            nc.gpsimd.dma_start(WVT_sb[:], WVT_d.rearrange("(c p) d -> p c d", p=P))
            for h in range(NH_LOC):
                nc.gpsimd.dma_start(V_sb[:, :, h, 64],
                                    vf_d.rearrange("(kc p) -> p kc", p=P))
            for qb in range(1, QB):
                nc.gpsimd.dma_start(
                    XT_sb[:, :, qb * 512:(qb + 1) * 512],
                    XT_d.rearrange("(c p) q -> p c q", p=P)
                        [:, :, qb * 512:(qb + 1) * 512])
            nc.gpsimd.dma_start(WOT_sb[:], WOT_d.rearrange("(t p) e -> p t e", p=P))

            def emit_kproj(t):
                for j0 in range(0, len(kb512), 2):
                    psk = ps.tile([P, 2, 512], f32, tag="s2", bufs=2,
                                  name=f"psk{t}_{j0}")
                    blks = kb512[j0:j0 + 2]
                    for jj, (k0, sz) in enumerate(blks):
                        for i in range(CC):
                            nc.tensor.matmul(
                                psk[:, jj, 0:sz],
                                WKT_sb[:, i, t * P:(t + 1) * P],
                                XKV_sb[:, i, k0:k0 + sz],
                                start=(i == 0), stop=(i == CC - 1))
                    lo = blks[0][0]
                    hi = blks[-1][0] + blks[-1][1]
                    nc.vector.tensor_scalar_add(
                        KT_sb[:, t, lo:hi],
                        psk.rearrange("p a b -> p (a b)")[:, 0:hi - lo],
                        bk_sb[:, t:t + 1])

            def emit_vproj(kc):
                # uses the avb/op1 1-bank slots (idle until AV/out-proj
                # start) so V never perturbs the scores/exp s3 rotation
                psv = ps.tile([P, 512], f32, tag="b1", bufs=4,
                              name=f"psv{kc}")
                for i in range(CC):
                    nc.tensor.matmul(
                        psv[:, 0:DH],
                        XKV_sb[:, i, kc * P:(kc + 1) * P],
                        WVT_sb[:, i, :],
                        start=(i == 0), stop=(i == CC - 1))
                nc.vector.tensor_copy(
                    V_sb[:, kc, :, 0:64],
                    psv[:, 0:DH].rearrange("p (h d) -> p h d", h=NH_LOC))

            qts = {}

            def emit_qproj(qb, bias_eng=None):
                q0 = qb * 512
                qt = qtp.tile([P, 2, 512], bf16, tag="qt", name=f"qt{qb}")
                qts[qb] = qt
                psq = ps.tile([P, 2, 512], f32, tag="s2", bufs=2)
                if bias_eng is None:
                    bias_eng = nc.vector
                for t in range(2):
                    for i in range(CC):
                        nc.tensor.matmul(
                            psq[:, t, :],
                            WQT_sb[:, i, t * P:(t + 1) * P],
                            XT_sb[:, i, q0:q0 + 512],
                            start=(i == 0), stop=(i == CC - 1))
                    bias_eng.tensor_scalar_add(
                        qt[:, t, :], psq[:, t, :], bq_sb[:, t:t + 1])

            ptts = {}

            def emit_scores(qb, h, groups):
                t, po = h // 2, (h % 2) * 64
                qt = qts[qb]
                if (qb, h) in ptts:
                    ptt = ptts[(qb, h)]
                else:
                    ptt = ptp.tile([P, KC, 512], bf16, tag="pt",
                                   name=f"pt{qb}_{h}")
                    ptts[(qb, h)] = ptt
                for kcs in groups:
                    pss = ps.tile([P, 2, 512], f32, tag="s2", bufs=2)
                    for i, kc in enumerate(kcs):
                        nc.tensor.matmul(
                            pss[:, i, :],
                            KT_sb[po:po + 64, t, kc * P:(kc + 1) * P],
                            qt[po:po + 64, t, :],
                            start=True, stop=True)
                    nc.scalar.activation(
                        ptt[:, kcs[0]:kcs[0] + len(kcs), :],
                        pss[:, 0:len(kcs), :],
                        mybir.ActivationFunctionType.Exp, scale=0.125)

            ots = {}

            def emit_av(qb, h):
                t, po = h // 2, (h % 2) * 64
                ptt = ptts.pop((qb, h))
                pso = ps.tile([P, 512], f32, tag="b1", bufs=4)
                for kc in range(KC):
                    nc.tensor.matmul(
                        pso[0:65, :],
                        V_sb[:, kc, h, :],
                        ptt[:, kc, :],
                        start=(kc == 0), stop=(kc == KC - 1))
                den = nrm.tile([1, 512], f32, tag="den")
                nc.vector.tensor_copy(den[:], pso[64:65, :])
                rec = nrm.tile([1, 512], f32, tag="rec")
                nc.vector.reciprocal_approx_fast(rec[:], den[:])
                recb = nrm.tile([64, 512], f32, tag="recb")
                nc.gpsimd.partition_broadcast(recb[:], rec[:], channels=64)
                if h == 0:
                    ot = otp.tile([P, 2, 512], bf16, tag="ot", name=f"ot{qb}")
                    ots[qb] = ot
                ot = ots[qb]
                nc.vector.tensor_mul(ot[po:po + 64, t, :],
                                     pso[0:64, :], recb[:])

            def emit_oproj_qc(qb, qc, tag="op1", copy_eng=None):
                q0 = qb * 512
                ot = ots[qb]
                ob = nrm.tile([P, 2, 512], bf16, tag="ob")
                if copy_eng is None:
                    copy_eng = nc.vector
                big = None
                for nb in range(2):
                    pso1 = ps.tile([P, 512], f32, tag="b1", bufs=4,
                                   name=f"op{qb}_{qc}_{nb}")
                    p3 = pso1[:]
                    for t in range(2):
                        nc.tensor.matmul(
                            p3,
                            ot[:, t, qc * P:(qc + 1) * P],
                            WOT_sb[:, t, nb * 512:(nb + 1) * 512],
                            start=(t == 0), stop=(t == 1))
                    copy_eng.tensor_copy(ob[:, nb, :], p3)
                nc.sync.dma_start(
                    OUT_d[q0 + qc * P:q0 + (qc + 1) * P, :], ob[:])

            # ---- software-pipelined emission over (qb, h) slots ----
            # per slot, PE order interleaves score groups with ready work
            # (AV of previous head, one out-proj chunk, Q-proj) so the PE
            # never head-of-line blocks waiting for an exp to free a slot.
            emit_kproj(0)
            emit_qproj(0, bias_eng=nc.vector)
            for s in range(NSLOT):
                qb, h = divmod(s, NH_LOC)
                emit_scores(qb, h, kc_groups[0:2])
                if s == 0:
                    for kc in range(KC):
                        emit_vproj(kc)
                if s >= 1:
                    pqb, ph = divmod(s - 1, NH_LOC)
                    emit_av(pqb, ph)
                emit_scores(qb, h, kc_groups[2:3])
                if s == 1:
                    emit_kproj(1)
                if qb >= 1:
                    emit_oproj_qc(qb - 1, h)
                emit_scores(qb, h, kc_groups[3:])
                if h == 1 and qb + 1 < QB:
                    emit_qproj(qb + 1)
            emit_av(QB - 1, NH_LOC - 1)
            # tail: spread the last block's out-proj over free psum tags and
            # copy engines (Act is idle by now)
            emit_oproj_qc(QB - 1, 0, copy_eng=nc.vector)
            emit_oproj_qc(QB - 1, 1, copy_eng=_ActCopy(nc))
            emit_oproj_qc(QB - 1, 2, copy_eng=_ActCopy(nc))
            emit_oproj_qc(QB - 1, 3, copy_eng=nc.vector)

    nc.compile()
    return nc


def kernel(X, mask, W_Q, b_Q, W_K, b_K, W_V, b_V, W_O, b_O):
    global last_results
    import concourse.mybir as mybir
    from concourse.bass_utils import run_bass_kernel_spmd

    f8 = mybir.dt.np(mybir.dt.float8e4)
    b16 = mybir.dt.np(mybir.dt.bfloat16)

    X = np.ascontiguousarray(X, dtype=np.float32)
    mask2 = np.asarray(mask).reshape(B, S) != 0
    counts = mask2.sum(axis=1)
    assert counts.min() >= 1
    SKV = max(P, int(math.ceil(counts.max() / P)) * P)

    XT = np.ascontiguousarray(X.transpose(0, 2, 1))          # (B, D, S)
    XKV = np.zeros((B, D_MODEL, SKV), dtype=np.float32)
    VF = np.zeros((B, SKV), dtype=np.float32)
    for b in range(B):
        idx = np.nonzero(mask2[b])[0]
        XKV[b, :, :len(idx)] = XT[b][:, idx]
        VF[b, :len(idx)] = 1.0

    nc = _build(SKV)

    in_maps = []
    for c in range(N_CORES):
        b, g = divmod(c, GROUPS)
        sl = slice(g * DH, (g + 1) * DH)
        in_maps.append({
            "xt": XT[b].astype(b16),
            "xkv": XKV[b].astype(b16),
            "wqt": np.ascontiguousarray(W_Q[sl, :].T).astype(b16),
            "wkt": np.ascontiguousarray(W_K[sl, :].T).astype(b16),
            "wvt": np.ascontiguousarray(W_V[sl, :].T).astype(b16),
            "wot": np.ascontiguousarray(W_O[:, sl].T).astype(b16),
            "bq": np.ascontiguousarray(b_Q[sl]),
            "bk": np.ascontiguousarray(b_K[sl]),
            "vf": VF[b].astype(b16),
        })

    trace_cores = None
    if os.environ.get("BASS_TRACE"):
        trace_cores = [int(x) for x in
                       os.environ.get("BASS_TRACE_CORES", "0").split(",")]
    res = run_bass_kernel_spmd(nc, in_maps, core_ids=list(range(N_CORES)),
                               trace_cores=trace_cores)
    last_results = res

    const = np.asarray(b_V, np.float64) @ np.asarray(W_O, np.float64).T \
        + np.asarray(b_O, np.float64)
    out = np.zeros((B, S, D_MODEL), dtype=np.float64)
    for c in range(N_CORES):
        b = c // GROUPS
        out[b] += res.results[c]["out"].astype(np.float64)
    out += const[None, None, :]
    return out.astype(np.float32)

